# revision 14
# baseline (speedup 1.0000x reference)
"""Self-contained Trainium2 Bass kernel for the routed-dense (MoE-style) layer.

Reference computation (per batch b, atom n):
    out[b,n,:] = tanh(W[ch[n]] @ x[b,n,:] + bias[ch[n]]) + x[b,n,:]
    returns (out, channels)

Strategy: expert-parallel — core c owns channel c outright (C == n_cores == 8),
so each core loads exactly one [OUT, IN] weight.  Rows (b, n) are routed to
cores by channel on the host; every core is padded to the max channel's row
count so all 8 cores run the identical SPMD program.  On device everything
lives in transposed layout out^T[o, row]: the x^T tiles feed both the matmul
(contraction over IN on partitions) and the residual add (IN == OUT).
Host-side x / out streams are block-packed so each block moves with a single
DMA whose per-partition runs are contiguous.

Two precision modes (MODE below):
  "fp16": x/w/out in fp16, fp32 PSUM accumulation (fastest, rel err ~5e-4)
  "f32r": x/w fp32 with float32r matmuls, fp16 out (rel err ~2.4e-4)
"""

import sys

for _p in ("/opt/trn_rl_repo", "/root/.axon_site/_ro/trn_rl_repo"):
    if _p not in sys.path:
        sys.path.insert(0, _p)

import numpy as np

MODE = "fp16"

B, N, IN, OUT, C = 64, 1024, 512, 512, 8
NCORES = 8
P = 128
KC = IN // P   # 4 contraction chunks
OC = OUT // P  # 4 output-partition chunks

_cache = {}


def _plan_blocks(rows, maxb):
    """Chunk `rows` columns into blocks <= maxb.  The first blocks ramp up
    (256, 512) so the opening DMA lands fast and the matmul stream starts
    early; the last block is kept small (256..512) to shorten the eviction
    tail.  All blocks are >= 256 so fp32r matmuls stay at full rate."""
    blocks = []
    off = 0
    rem = rows
    for warm in (128, 256, 512):
        if maxb > 512 and rem >= warm + 512:
            blocks.append((off, warm))
            off += warm
            rem -= warm
    while rem > 0:
        if rem > maxb + 512:
            take = maxb
        elif rem > maxb:
            take = (rem // 2) + (rem & 1)
        else:
            take = rem
        blocks.append((off, take))
        off += take
        rem -= take
    return blocks


def _build_program(rows, mode):
    import concourse.bacc as bacc
    import concourse.tile as tile
    import concourse.mybir as mybir
    from contextlib import ExitStack

    F32 = mybir.dt.float32
    F16 = mybir.dt.float16
    F32R = mybir.dt.float32r
    MM_DT = F16 if mode == "fp16" else F32R
    RES_DT = F16 if mode == "fp16" else F32
    MAXB = 1024 if mode == "fp16" else 512

    blocks = _plan_blocks(rows, MAXB)

    nc = bacc.Bacc("TRN2", target_bir_lowering=False, debug=False)
    # block-packed x^T: [128, 4 * rows], block b occupying [:, 4*off : 4*(off+bs)]
    # with inner layout [kc][col]
    xt_ext = nc.dram_tensor("xt", [P, KC * rows], RES_DT, kind="ExternalInput")
    # weight^T for this core's channel, packed [128, KC * OUT] ([kc][o])
    wt_ext = nc.dram_tensor("wt", [P, KC * OUT], RES_DT, kind="ExternalInput")
    b_ext = nc.dram_tensor("b", [P, OC], F32, kind="ExternalInput")
    # block-packed out^T: [128, 4 * rows], inner layout [oc][col]
    yt_ext = nc.dram_tensor("yt", [P, OC * rows], F16, kind="ExternalOutput")

    with tile.TileContext(nc) as tc:
        with ExitStack() as ctx:
            wpool = ctx.enter_context(tc.tile_pool(name="w", bufs=1))
            bpool = ctx.enter_context(tc.tile_pool(name="bias", bufs=1))
            xpool = ctx.enter_context(tc.tile_pool(name="x", bufs=8))
            apool = ctx.enter_context(tc.tile_pool(name="a", bufs=8))
            opool = ctx.enter_context(tc.tile_pool(name="o", bufs=4))
            n_pbuf = 4 if MAXB == 1024 else 8
            ppool = ctx.enter_context(tc.tile_pool(name="p", bufs=n_pbuf, space="PSUM"))

            w_sb = wpool.tile([P, KC * OUT], MM_DT)
            nc.sync.dma_start(w_sb[:], wt_ext[:].bitcast(MM_DT))
            b_sb = bpool.tile([P, OC], F32)
            nc.sync.dma_start(b_sb[:], b_ext[:])

            for bi, (off, bs) in enumerate(blocks):
                halves = [(h, min(512, bs - h)) for h in range(0, bs, 512)]
                xt = xpool.tile([P, KC * MAXB], MM_DT, tag="xt")
                nc.sync.dma_start(
                    xt[:, : KC * bs],
                    xt_ext[:, KC * off : KC * (off + bs)].bitcast(MM_DT),
                )
                o_sb = opool.tile([P, OC * MAXB], F16, tag="out")
                for oc in range(OC):
                    psum = ppool.tile([P, MAXB], F32, tag="ps")
                    for h, hs in halves:
                        for kc in range(KC):
                            nc.tensor.matmul(
                                psum[:, h : h + hs],
                                lhsT=w_sb[:, kc * OUT + oc * P : kc * OUT + (oc + 1) * P],
                                rhs=xt[:, kc * bs + h : kc * bs + h + hs],
                                start=(kc == 0),
                                stop=(kc == KC - 1),
                            )
                    a_sb = apool.tile([P, MAXB], F16, tag="act")
                    nc.scalar.activation(
                        a_sb[:, :bs],
                        psum[:, :bs],
                        mybir.ActivationFunctionType.Tanh,
                        bias=b_sb[:, oc : oc + 1],
                    )
                    nc.vector.tensor_add(
                        o_sb[:, oc * bs : (oc + 1) * bs],
                        a_sb[:, :bs],
                        xt[:, oc * bs : (oc + 1) * bs].bitcast(RES_DT),
                    )
                nc.gpsimd.dma_start(
                    yt_ext[:, OC * off : OC * (off + bs)], o_sb[:, : OC * bs]
                )

    nc.compile()
    return nc, blocks


def _get_program(rows, mode):
    key = (rows, mode)
    if key not in _cache:
        _cache[key] = _build_program(rows, mode)
    return _cache[key]


def kernel(x, channels, weight, bias, _want_trace=False):
    from concourse.bass_utils import run_bass_kernel_spmd

    x = np.asarray(x)
    ch_in = channels
    ch = np.asarray(channels).astype(np.int64)
    weight = np.asarray(weight, dtype=np.float32)
    bias = np.asarray(bias, dtype=np.float32)

    in_np_dt = np.float16 if MODE == "fp16" else np.float32

    counts = np.bincount(ch, minlength=C)
    rows = int(counts.max()) * B
    nc, blocks = _get_program(rows, MODE)


    # ---- host-side routing / sharding ----
    # column order per core: this core's channel's atoms, batch innermost,
    # padded to `rows` by repeating the first column
    atom_ids = [np.where(ch == c)[0] for c in range(C)]
    x_nb = np.ascontiguousarray(x.transpose(2, 1, 0)).reshape(IN, N * B)
    if MODE == "fp16":
        x_nb = x_nb.astype(np.float16)

    core_cols = []
    for c in range(C):
        cols = (atom_ids[c][:, None] * B + np.arange(B)[None, :]).ravel()
        if len(cols) < rows:
            pad = np.zeros(rows - len(cols), dtype=np.int64)
            if len(cols):
                pad[:] = cols[0]
            cols = np.concatenate([cols, pad])
        core_cols.append(cols)

    in_maps = []
    for c in range(C):
        xt = x_nb[:, core_cols[c]]  # [IN, rows]
        # block-pack: [128, 4 * rows], block b -> [:, 4*off:4*(off+bs)] = [kc][col]
        xt_packed = np.empty((P, KC * rows), dtype=in_np_dt)
        xt4 = xt.reshape(KC, P, rows)
        for off, bs in blocks:
            xt_packed[:, KC * off : KC * (off + bs)] = (
                xt4[:, :, off : off + bs].transpose(1, 0, 2).reshape(P, KC * bs)
            )
        wt = np.ascontiguousarray(
            weight[c].T.reshape(KC, P, OUT).transpose(1, 0, 2).reshape(P, KC * OUT)
        ).astype(in_np_dt)
        b_in = np.ascontiguousarray(bias[c].reshape(OC, P).T)
        in_maps.append({"xt": xt_packed, "wt": wt, "b": b_in})

    last_err = None
    for _attempt in range(3):
        try:
            res = run_bass_kernel_spmd(
                nc, in_maps, list(range(NCORES)), trace=_want_trace
            )
            break
        except Exception as e:  # transient NRT device errors on cold NEFFs
            last_err = e
            import time as _time

            _time.sleep(2.0)
    else:
        raise last_err

    # ---- unshard ----
    out_nb = np.empty((N * B, OUT), dtype=np.float32)
    for c in range(C):
        yt_packed = res.results[c]["yt"].astype(np.float32)  # [128, OC*rows] packed
        n_real = len(atom_ids[c]) * B
        for off, bs in blocks:
            if off >= n_real:
                break
            take = min(bs, n_real - off)
            blk = (
                yt_packed[:, OC * off : OC * (off + bs)]
                .reshape(P, OC, bs)
                .transpose(1, 0, 2)
                .reshape(OUT, bs)
            )
            out_nb[core_cols[c][off : off + take]] = blk[:, :take].T
    out = np.ascontiguousarray(out_nb.reshape(N, B, OUT).transpose(1, 0, 2))

    if _want_trace:
        kernel._last_results = res
    return (out, ch_in)


# revision 15
# speedup vs baseline: 1.0659x; 1.0659x over previous
"""Self-contained Trainium2 Bass kernel for the routed-dense (MoE-style) layer.

Reference computation (per batch b, atom n):
    out[b,n,:] = tanh(W[ch[n]] @ x[b,n,:] + bias[ch[n]]) + x[b,n,:]
    returns (out, channels)

Strategy: expert-parallel — core c owns channel c outright (C == n_cores == 8),
so each core loads exactly one [OUT, IN] weight.  Rows (b, n) are routed to
cores by channel on the host; every core is padded to the max channel's row
count so all 8 cores run the identical SPMD program.  On device everything
lives in transposed layout out^T[o, row]: the x^T tiles feed both the matmul
(contraction over IN on partitions) and the residual add (IN == OUT).
Host-side x / out streams are block-packed so each block moves with a single
DMA whose per-partition runs are contiguous.

Two precision modes (MODE below):
  "fp16": x/w/out in fp16, fp32 PSUM accumulation (fastest, rel err ~5e-4)
  "f32r": x/w fp32 with float32r matmuls, fp16 out (rel err ~2.4e-4)
"""

import sys

for _p in ("/opt/trn_rl_repo", "/root/.axon_site/_ro/trn_rl_repo"):
    if _p not in sys.path:
        sys.path.insert(0, _p)

import numpy as np

MODE = "fp16"

B, N, IN, OUT, C = 64, 1024, 512, 512, 8
NCORES = 8
P = 128
KC = IN // P   # 4 contraction chunks
OC = OUT // P  # 4 output-partition chunks

_cache = {}


def _plan_blocks(rows, maxb):
    """Chunk `rows` columns into blocks <= maxb.  The first blocks ramp up
    (256, 512) so the opening DMA lands fast and the matmul stream starts
    early; the last block is kept small (256..512) to shorten the eviction
    tail.  All blocks are >= 256 so fp32r matmuls stay at full rate."""
    blocks = []
    off = 0
    rem = rows
    for warm in (256, 512):
        if maxb > 512 and rem >= warm + 512:
            blocks.append((off, warm))
            off += warm
            rem -= warm
    while rem > 0:
        if rem > maxb + 512:
            take = maxb
        elif rem > maxb:
            take = (rem // 2) + (rem & 1)
        else:
            take = rem
        blocks.append((off, take))
        off += take
        rem -= take
    return blocks


def _build_program(rows, mode):
    import concourse.bacc as bacc
    import concourse.tile as tile
    import concourse.mybir as mybir
    from contextlib import ExitStack

    F32 = mybir.dt.float32
    F16 = mybir.dt.float16
    F32R = mybir.dt.float32r
    MM_DT = F16 if mode == "fp16" else F32R
    RES_DT = F16 if mode == "fp16" else F32
    MAXB = 1024 if mode == "fp16" else 512

    blocks = _plan_blocks(rows, MAXB)

    nc = bacc.Bacc("TRN2", target_bir_lowering=False, debug=False)
    # block-packed x^T: [128, 4 * rows], block b occupying [:, 4*off : 4*(off+bs)]
    # with inner layout [kc][col]
    xt_ext = nc.dram_tensor("xt", [P, KC * rows], RES_DT, kind="ExternalInput")
    # weight^T for this core's channel, packed [128, KC * OUT] ([kc][o])
    wt_ext = nc.dram_tensor("wt", [P, KC * OUT], RES_DT, kind="ExternalInput")
    b_ext = nc.dram_tensor("b", [P, OC], F32, kind="ExternalInput")
    # block-packed out^T: [128, 4 * rows], inner layout [oc][col]
    yt_ext = nc.dram_tensor("yt", [P, OC * rows], F16, kind="ExternalOutput")

    with tile.TileContext(nc) as tc:
        with ExitStack() as ctx:
            wpool = ctx.enter_context(tc.tile_pool(name="w", bufs=1))
            bpool = ctx.enter_context(tc.tile_pool(name="bias", bufs=1))
            xpool = ctx.enter_context(tc.tile_pool(name="x", bufs=8))
            apool = ctx.enter_context(tc.tile_pool(name="a", bufs=8))
            opool = ctx.enter_context(tc.tile_pool(name="o", bufs=4))
            n_pbuf = 4 if MAXB == 1024 else 8
            ppool = ctx.enter_context(tc.tile_pool(name="p", bufs=n_pbuf, space="PSUM"))

            w_sb = wpool.tile([P, KC * OUT], MM_DT)
            nc.sync.dma_start(w_sb[:], wt_ext[:].bitcast(MM_DT))
            b_sb = bpool.tile([P, OC], F32)
            nc.sync.dma_start(b_sb[:], b_ext[:])

            for bi, (off, bs) in enumerate(blocks):
                halves = [(h, min(512, bs - h)) for h in range(0, bs, 512)]
                xt = xpool.tile([P, KC * MAXB], MM_DT, tag="xt")
                nc.sync.dma_start(
                    xt[:, : KC * bs],
                    xt_ext[:, KC * off : KC * (off + bs)].bitcast(MM_DT),
                )
                o_sb = opool.tile([P, OC * MAXB], F16, tag="out")
                for oc in range(OC):
                    psum = ppool.tile([P, MAXB], F32, tag="ps")
                    for h, hs in halves:
                        for kc in range(KC):
                            nc.tensor.matmul(
                                psum[:, h : h + hs],
                                lhsT=w_sb[:, kc * OUT + oc * P : kc * OUT + (oc + 1) * P],
                                rhs=xt[:, kc * bs + h : kc * bs + h + hs],
                                start=(kc == 0),
                                stop=(kc == KC - 1),
                            )
                    a_sb = apool.tile([P, MAXB], F16, tag="act")
                    nc.scalar.activation(
                        a_sb[:, :bs],
                        psum[:, :bs],
                        mybir.ActivationFunctionType.Tanh,
                        bias=b_sb[:, oc : oc + 1],
                    )
                    nc.vector.tensor_add(
                        o_sb[:, oc * bs : (oc + 1) * bs],
                        a_sb[:, :bs],
                        xt[:, oc * bs : (oc + 1) * bs].bitcast(RES_DT),
                    )
                nc.gpsimd.dma_start(
                    yt_ext[:, OC * off : OC * (off + bs)], o_sb[:, : OC * bs]
                )

    nc.compile()
    return nc, blocks


def _get_program(rows, mode):
    key = (rows, mode)
    if key not in _cache:
        _cache[key] = _build_program(rows, mode)
    return _cache[key]


def kernel(x, channels, weight, bias, _want_trace=False):
    from concourse.bass_utils import run_bass_kernel_spmd

    x = np.asarray(x)
    ch_in = channels
    ch = np.asarray(channels).astype(np.int64)
    weight = np.asarray(weight, dtype=np.float32)
    bias = np.asarray(bias, dtype=np.float32)

    in_np_dt = np.float16 if MODE == "fp16" else np.float32

    counts = np.bincount(ch, minlength=C)
    rows = int(counts.max()) * B
    nc, blocks = _get_program(rows, MODE)


    # ---- host-side routing / sharding ----
    # column order per core: this core's channel's atoms, batch innermost,
    # padded to `rows` by repeating the first column
    atom_ids = [np.where(ch == c)[0] for c in range(C)]
    x_nb = np.ascontiguousarray(x.transpose(2, 1, 0)).reshape(IN, N * B)
    if MODE == "fp16":
        x_nb = x_nb.astype(np.float16)

    core_cols = []
    for c in range(C):
        cols = (atom_ids[c][:, None] * B + np.arange(B)[None, :]).ravel()
        if len(cols) < rows:
            pad = np.zeros(rows - len(cols), dtype=np.int64)
            if len(cols):
                pad[:] = cols[0]
            cols = np.concatenate([cols, pad])
        core_cols.append(cols)

    in_maps = []
    for c in range(C):
        xt = x_nb[:, core_cols[c]]  # [IN, rows]
        # block-pack: [128, 4 * rows], block b -> [:, 4*off:4*(off+bs)] = [kc][col]
        xt_packed = np.empty((P, KC * rows), dtype=in_np_dt)
        xt4 = xt.reshape(KC, P, rows)
        for off, bs in blocks:
            xt_packed[:, KC * off : KC * (off + bs)] = (
                xt4[:, :, off : off + bs].transpose(1, 0, 2).reshape(P, KC * bs)
            )
        wt = np.ascontiguousarray(
            weight[c].T.reshape(KC, P, OUT).transpose(1, 0, 2).reshape(P, KC * OUT)
        ).astype(in_np_dt)
        b_in = np.ascontiguousarray(bias[c].reshape(OC, P).T)
        in_maps.append({"xt": xt_packed, "wt": wt, "b": b_in})

    last_err = None
    for _attempt in range(3):
        try:
            res = run_bass_kernel_spmd(
                nc, in_maps, list(range(NCORES)), trace=_want_trace
            )
            break
        except Exception as e:  # transient NRT device errors on cold NEFFs
            last_err = e
            import time as _time

            _time.sleep(2.0)
    else:
        raise last_err

    # ---- unshard ----
    out_nb = np.empty((N * B, OUT), dtype=np.float32)
    for c in range(C):
        yt_packed = res.results[c]["yt"].astype(np.float32)  # [128, OC*rows] packed
        n_real = len(atom_ids[c]) * B
        for off, bs in blocks:
            if off >= n_real:
                break
            take = min(bs, n_real - off)
            blk = (
                yt_packed[:, OC * off : OC * (off + bs)]
                .reshape(P, OC, bs)
                .transpose(1, 0, 2)
                .reshape(OUT, bs)
            )
            out_nb[core_cols[c][off : off + take]] = blk[:, :take].T
    out = np.ascontiguousarray(out_nb.reshape(N, B, OUT).transpose(1, 0, 2))

    if _want_trace:
        kernel._last_results = res
    return (out, ch_in)
